# revision 54
# baseline (speedup 1.0000x reference)
"""Distributed Bass kernel: causal multi-head attention with RoPE (v2).

Full op:  x[2,2048,2048] -> attention(16 heads, RoPE, causal) @ wo.T
Sharding: core = b*4 + j  (b in {0,1} batch, j in {0..3} head-group)
  - core owns batch b, heads 4j..4j+3 (512 of the 2048 hidden dims)

v2 structure (single fused pipeline, emission-interleaved):
  - host packs x/weights into DMA-contiguous SBUF layouts
  - qk(h0) + v projection interleaved, paced by x-slice DMA arrival
  - attention(h) runs with qk(h+1) matmuls as PE filler (keeps PE warm)
  - softmax denominators: ones[128x128] matmul broadcasts column-sums to
    all partitions; reciprocal = ACT Exp(-Ln(x)) (same ACT table set as
    the softmax Exp -> no table thrash); one DVE mult normalizes
  - AllGather of yT split per (head, sq-half): 8 smaller earlier gathers
  - phase 3 consumes gathered halves in order so the last gather hides
Compute dtype bf16 (f32 accumulation in PSUM); inputs converted on host.
"""

import math
import os
import sys

for _p in ("/opt/trn_rl_repo",):
    if _p not in sys.path:
        sys.path.insert(0, _p)

import ml_dtypes
import numpy as np

import concourse.bass as bass  # noqa: F401
import concourse.mybir as mybir
import concourse.tile as tile
from concourse import bacc
from concourse.bass_utils import run_bass_kernel_spmd

BF16 = mybir.dt.bfloat16
F32 = mybir.dt.float32
NPBF16 = ml_dtypes.bfloat16

B, S, D = 2, 2048, 2048
H, HD = 16, 128
BASE = 10000
NCORES = 8
GROUPS = [[0, 1, 2, 3], [4, 5, 6, 7]]
HPC = 4            # heads per core
DPC = HPC * HD     # 512 hidden dims per core
KC = D // 128      # 16 contraction chunks
NS = S // 512      # 4 seq slices of 512
SCALE = 1.0 / math.sqrt(HD)
NEG = -30000.0

ExpF = mybir.ActivationFunctionType.Exp
LnF = mybir.ActivationFunctionType.Ln

_CACHE = {}

LAST_EXEC_NS = None
LAST_TRACE = None


def _install_ntff_hook():
    """The image's antenv lacks axon_hooks; bass_utils hard-imports it when
    trace=True. Register the boot module's ctypes hook under that name."""
    try:
        import antenv.axon_hooks  # noqa: F401
        return True
    except ImportError:
        pass
    try:
        import types

        import antenv
        from trn_agent_boot.trn_boot import _ntff_profile_via_ctypes

        mod = types.ModuleType("antenv.axon_hooks")
        _hook = [None]
        mod.set_axon_ntff_profile_hook = lambda h: _hook.__setitem__(0, h)
        mod.get_axon_ntff_profile_hook = lambda: _hook[0]
        sys.modules["antenv.axon_hooks"] = mod
        antenv.axon_hooks = mod
        mod.set_axon_ntff_profile_hook(
            _ntff_profile_via_ctypes("/opt/axon/libaxon_pjrt.so")
        )
        return True
    except Exception:
        return False


def _patch_act_tables():
    """Restrict the activation-table catalog to the single set holding both
    Exp and Ln (natural_log_exp_and_others). Without this the table-load
    pass alternates Exp/Ln sets every softmax epilogue (~2.6us each)."""
    from concourse import bacc as bacc_mod
    if getattr(bacc_mod, "_act_patch", False):
        return
    orig = bacc_mod.get_activation_tables

    def _gat(arch):
        t = orig(arch)
        E = mybir.ActivationFunctionType.Exp
        L = mybir.ActivationFunctionType.Ln
        keep = {k: v for k, v in t.items() if E in v and L in v}
        return keep or t

    bacc_mod.get_activation_tables = _gat
    bacc_mod._act_patch = True


def _patch_ldw_opt():
    """Flip walrus's hardcoded --enable-ldw-opt=false to true (LDWEIGHTS
    scheduling optimization; worth ~46ns/matmul if it works)."""
    from concourse import bass_utils as bu
    if getattr(bu, "_ldw_patch", False):
        return
    orig = bu.run_command

    def run2(argv, **kw):
        argv = ["--enable-ldw-opt=true" if a == "--enable-ldw-opt=false" else a
                for a in argv]
        return orig(argv, **kw)

    bu.run_command = run2
    bu._ldw_patch = True


def _build():
    if os.environ.get("LDW_OPT", "0") == "1":
        _patch_ldw_opt()
    nc = bacc.Bacc(None, target_bir_lowering=False, num_devices=NCORES)

    # packed params (see kernel() for layouts)
    xP = nc.declare_dram_parameter("xP", [128, NS * KC * 512], BF16, isOutput=False)
    wqP = nc.declare_dram_parameter("wqP", [128, HPC * KC * 128], BF16, isOutput=False)
    wkP = nc.declare_dram_parameter("wkP", [128, HPC * KC * 128], BF16, isOutput=False)
    wvP = nc.declare_dram_parameter("wvP", [128, KC * 512], BF16, isOutput=False)
    woP = nc.declare_dram_parameter("woP", [128, H * 512], BF16, isOutput=False)
    cosE = nc.declare_dram_parameter("cosE", [HD, S], BF16, isOutput=False)
    sinE = nc.declare_dram_parameter("sinE", [HD, S], BF16, isOutput=False)
    pswap = nc.declare_dram_parameter("pswap", [128, 128], BF16, isOutput=False)
    btri = nc.declare_dram_parameter("btri", [128, 128], BF16, isOutput=False)
    ident = nc.declare_dram_parameter("ident", [128, 128], BF16, isOutput=False)
    ones2 = nc.declare_dram_parameter("ones2", [128, 128], BF16, isOutput=False)
    out = nc.declare_dram_parameter("out", [S, DPC], F32, isOutput=True)

    with tile.TileContext(nc) as tc:
        with (
            tc.tile_pool(name="consts", bufs=1) as cpool,
            tc.tile_pool(name="stream", bufs=2) as spool,
            tc.tile_pool(name="work", bufs=1) as wpool,
            tc.tile_pool(name="dram", bufs=1, space="DRAM") as dpool,
            tc.tile_pool(name="psA", bufs=2, space="PSUM") as psA,   # psc [128,1024]
            tc.tile_pool(name="psB", bufs=1, space="PSUM") as psB,   # pyt [128,512]
            tc.tile_pool(name="psC", bufs=1, space="PSUM") as psC,   # psm [128,512]
            tc.tile_pool(name="psD", bufs=2, space="PSUM") as psD,   # pq/sw [128,512]
        ):
            # ---- tiny dummy gather: absorbs collective-engine startup ----
            warm_in = dpool.tile([128, 2], BF16, tag="warm_in", name="warm_in")
            warm_out = dpool.tile([512, 2], BF16, tag="warm_out", name="warm_out")
            nc.gpsimd.dma_start(out=warm_in[:], in_=ones2[:, 0:2])
            nc.gpsimd.collective_compute(
                "AllGather",
                mybir.AluOpType.bypass,
                replica_groups=GROUPS,
                ins=[warm_in[:].opt()],
                outs=[warm_out[:].opt()],
            )

            # ---- input DMAs ----
            # x rides the sync ring alone (parallel rings measured slower:
            # ~87GB/s each, gpsimd SWDGE starts late). Slice 0 is split so
            # its first half (k-chunks 0-7) lands ~12us in.
            bigA_cm = tc.tile_pool(name="bigA", bufs=1)
            bigA = bigA_cm.__enter__()
            xta = bigA.tile([128, NS * KC * 512], BF16, tag="xta", name="xta")
            wva = bigA.tile([128, KC * 512], BF16, tag="wva", name="wva")
            nc.sync.dma_start(out=xta[:, 0:1024], in_=xP[:, 0:1024])
            nc.sync.dma_start(out=xta[:, 1024:2048], in_=xP[:, 1024:2048])
            nc.sync.dma_start(out=xta[:, 2048:4096], in_=xP[:, 2048:4096])
            nc.sync.dma_start(out=xta[:, 4096:6144], in_=xP[:, 4096:6144])
            nc.sync.dma_start(out=xta[:, 6144:8192], in_=xP[:, 6144:8192])
            for n in range(1, NS):
                nc.sync.dma_start(
                    out=xta[:, n * 8192:(n + 1) * 8192],
                    in_=xP[:, n * 8192:(n + 1) * 8192],
                )

            # vector queue: per-head q/k weights + consts; wo last
            whq, whk = {}, {}

            def dma_wqk(h):
                # bufs=4: all four heads resident; a bufs<4 rotation would
                # park this DMA's wait in the DVE FIFO and deadlock against
                # the rope tails that retire the previous head's weights.
                whq[h] = spool.tile([128, KC * 128], BF16, tag="whq", bufs=4,
                                    name=f"whq{h}")
                nc.scalar.dma_start(
                    out=whq[h][:], in_=wqP[:, h * 2048:(h + 1) * 2048]
                )
                whk[h] = spool.tile([128, KC * 128], BF16, tag="whk", bufs=4,
                                    name=f"whk{h}")
                nc.scalar.dma_start(
                    out=whk[h][:], in_=wkP[:, h * 2048:(h + 1) * 2048]
                )

            dma_wqk(0)
            nc.scalar.dma_start(out=wva[:], in_=wvP[:, :])
            p_t = cpool.tile([128, 128], BF16, tag="pswap", name="pswap")
            nc.scalar.dma_start(out=p_t[:], in_=pswap[:, :])
            cos_t = cpool.tile([HD, S], BF16, tag="cos", name="cos")
            nc.scalar.dma_start(out=cos_t[:], in_=cosE[:, :])
            sin_t = cpool.tile([HD, S], BF16, tag="sin", name="sin")
            nc.scalar.dma_start(out=sin_t[:], in_=sinE[:, :])
            btri_t = cpool.tile([128, 128], BF16, tag="btri", name="btri")
            nc.scalar.dma_start(out=btri_t[:], in_=btri[:, :])
            id_t = cpool.tile([128, 128], BF16, tag="ident", name="ident")
            nc.scalar.dma_start(out=id_t[:], in_=ident[:, :])
            ones_t = cpool.tile([128, 128], BF16, tag="ones2", name="ones2")
            nc.scalar.dma_start(out=ones_t[:], in_=ones2[:, :])
            dma_wqk(1)
            dma_wqk(2)
            dma_wqk(3)
            woa = cpool.tile([128, H * 512], BF16, tag="woa", name="woa")
            nc.scalar.dma_start(out=woa[:], in_=woP[:, :])

            # collective DRAM buffers, per (head, sq-half)
            bin_hh = [[dpool.tile([128, 1024], BF16, tag=f"bin{h}_{v}",
                                  name=f"bin{h}_{v}") for v in range(2)]
                      for h in range(HPC)]
            bout_hh = [[dpool.tile([512, 1024], BF16, tag=f"bout{h}_{v}",
                                   name=f"bout{h}_{v}") for v in range(2)]
                       for h in range(HPC)]

            vva = cpool.tile([128, KC * 512], BF16, tag="vva", name="vva")
            qTt, kTt = {}, {}

            # ---------------- emission helpers ----------------
            def rope_tail(pq, dst, n, sw_pool, sw_tag):
                """pq: [128,512] psum with pre-rope head tile (transposed
                layout); writes rope'd result into dst[:, n*512:+512]."""
                raw = wpool.tile([128, 512], BF16, tag="raw", bufs=2, name="raw")
                nc.vector.tensor_copy(raw[:], pq[:])
                sw = sw_pool.tile([128, 512], F32, tag=sw_tag, name="sw")
                nc.tensor.matmul(sw[:], p_t[:], raw[:], start=True, stop=True)
                t1 = wpool.tile([128, 512], BF16, tag="t1", bufs=2, name="t1")
                nc.vector.tensor_tensor(
                    t1[:], raw[:], cos_t[:, n * 512:(n + 1) * 512],
                    mybir.AluOpType.mult,
                )
                t2 = wpool.tile([128, 512], BF16, tag="t2", bufs=2, name="t2")
                nc.vector.tensor_tensor(
                    t2[:], sw[:], sin_t[:, n * 512:(n + 1) * 512],
                    mybir.AluOpType.mult,
                )
                nc.vector.tensor_tensor(
                    dst[:, n * 512:(n + 1) * 512], t1[:], t2[:],
                    mybir.AluOpType.add,
                )

            def qk_closures(h, sw_pool=None, sw_tag="pq"):
                """Projection of q,k for head h as a list of closures:
                per n: two 16-MM chunks (q/k chains interleaved so matmuls
                alternate PSUM banks and drain overlaps fill) + two rope
                tails."""
                if sw_pool is None:
                    sw_pool = psD
                qTt[h] = spool.tile([128, S], BF16, tag="qT", name=f"qT{h}")
                kTt[h] = spool.tile([128, S], BF16, tag="kT", name=f"kT{h}")
                ops = []
                state = {}

                def mk_mms(n, klo, khi):
                    def go():
                        if klo == 0:
                            state[("q", n)] = psD.tile([128, 512], F32,
                                                       tag="pq", name="pq")
                            state[("k", n)] = psD.tile([128, 512], F32,
                                                       tag="pq", name="pk")
                        for k in range(klo, khi):
                            for wt, key in ((whq[h], "q"), (whk[h], "k")):
                                nc.tensor.matmul(
                                    state[(key, n)][:],
                                    wt[:, k * 128:(k + 1) * 128],
                                    xta[:, n * 8192 + k * 512:
                                        n * 8192 + k * 512 + 512],
                                    start=(k == 0), stop=(k == KC - 1),
                                )
                    return go

                def mk_tail(key, dst, n):
                    def go():
                        rope_tail(state.pop((key, n)), dst, n, sw_pool, sw_tag)
                    return go

                for n in range(NS):
                    if n == 0 and h == 0:
                        # finer pieces so head-0 matmuls start as soon as
                        # the first 256KB x piece lands
                        mms = [mk_mms(n, 0, 2), mk_mms(n, 2, 4),
                               mk_mms(n, 4, 8), mk_mms(n, 8, 12),
                               mk_mms(n, 12, KC)]
                    else:
                        mms = [mk_mms(n, 0, 8), mk_mms(n, 8, KC)]
                    ops.append((mms, mk_tail("q", qTt[h], n),
                                mk_tail("k", kTt[h], n)))
                return ops

            def v_pair(j):
                """v projection for s-chunk pair (2j, 2j+1) -> vva; halves
                interleaved in the k-loop so matmuls alternate PSUM banks.
                The psum->sbuf copy rides the otherwise idle ACT engine to
                keep the DVE FIFO free for rope tails."""
                pv = psA.tile([128, 1024], F32, tag="psc", name="pv")
                for k in range(KC):
                    for half in range(2):
                        m = 2 * j + half
                        n, mo = m // 4, (m % 4) * 128
                        nc.tensor.matmul(
                            pv[:, half * 512:half * 512 + 512],
                            xta[:, n * 8192 + k * 512 + mo:
                                n * 8192 + k * 512 + mo + 128],
                            wva[:, k * 512:(k + 1) * 512],
                            start=(k == 0), stop=(k == KC - 1),
                        )
                nc.scalar.copy(vva[:, 2 * j * 512:(2 * j + 2) * 512], pv[:])

            # ---------------- qk(h0) + v, paced by x-slice arrival ----------
            # Per slice: qk matmuls, then the two v pairs woven around the
            # rope tails so the P-swap matmuls never wait on the DVE copies.
            # sw borrows the idle psm bank during h0.
            qk0 = qk_closures(0, sw_pool=psC, sw_tag="psm")
            for n in range(NS):
                mms, tq, tk = qk0[n]
                for m_ in mms:
                    m_()
                v_pair(2 * n)
                tq()
                v_pair(2 * n + 1)
                tk()

            # ---------------- attention(h) with qk(h+1) filler --------------
            def attention(h, filler):
                fi = [0]

                def drain(k):
                    while k > 0 and fi[0] < len(filler):
                        filler[fi[0]]()
                        fi[0] += 1
                        k -= 1

                qT, kT = qTt[h], kTt[h]
                for slc in range(NS):
                    sq0 = slc * 512
                    pyt = psB.tile([128, 512], F32, tag="pyt", name="pyt")
                    psm = psC.tile([128, 512], F32, tag="psm", name="psm")
                    nchunks = (slc + 1) * 4
                    pend = None
                    for kp in range(0, nchunks, 2):
                        psc = psA.tile([128, 1024], F32, tag="psc", name="psc")
                        dlts = []
                        for half in range(2):
                            kk = kp + half
                            off = half * 512
                            diag = kk >= slc * 4
                            dlt = (kk - slc * 4) * 128 if diag else 0
                            dlts.append(dlt)
                            nc.tensor.matmul(
                                psc[:, off + dlt:off + 512],
                                kT[:, kk * 128:(kk + 1) * 128],
                                qT[:, sq0 + dlt:sq0 + 512],
                                start=True, stop=not diag,
                            )
                            if diag:
                                nc.tensor.matmul(
                                    psc[:, off + dlt:off + dlt + 128],
                                    id_t[:], btri_t[:],
                                    start=False, stop=True,
                                )
                        pb = wpool.tile([128, 1024], BF16, tag="pb", bufs=6,
                                        name="pb")
                        nc.scalar.activation(
                            pb[:, dlts[0]:1024], psc[:, dlts[0]:1024],
                            ExpF, scale=SCALE,
                        )
                        drain(1)
                        if pend is not None:
                            pend()
                        dl0, dl1 = dlts

                        def mk_pend(kp=kp, pb=pb, dl0=dl0, dl1=dl1):
                            def go():
                                for half, dlt in ((0, dl0), (1, dl1)):
                                    kk = kp + half
                                    off = half * 512
                                    nc.tensor.matmul(
                                        psm[:, dlt:512], ones_t[:],
                                        pb[:, off + dlt:off + 512],
                                        start=(kk == 0), stop=(kk == nchunks - 1),
                                    )
                                    nc.tensor.matmul(
                                        pyt[:, dlt:512],
                                        vva[:, kk * 512 + h * 128:
                                            kk * 512 + (h + 1) * 128],
                                        pb[:, off + dlt:off + 512],
                                        start=(kk == 0), stop=(kk == nchunks - 1),
                                    )
                            return go

                        pend = mk_pend()
                    pend()
                    # epilogue: ACT copies free the psum banks fast; the
                    # reciprocal is a bitcast-seeded Newton iteration in SBUF
                    # on the DVE (plain ALU ops only — DVE divide/reciprocal
                    # and ACT Ln are unusable here: table-op lowering breaks
                    # and Exp<->Ln table switches cost 2.6us each).
                    yts = wpool.tile([128, 512], F32, tag="yts", bufs=2,
                                     name="yts")
                    nc.scalar.copy(yts[:], pyt[:])
                    sms = wpool.tile([128, 512], F32, tag="sms", bufs=2,
                                     name="sms")
                    nc.scalar.copy(sms[:], psm[:])
                    r = wpool.tile([128, 512], F32, tag="rcpn", bufs=2,
                                   name="rcpn")
                    nc.vector.tensor_scalar(
                        r[:].bitcast(mybir.dt.int32),
                        sms[:].bitcast(mybir.dt.int32),
                        0x7EF311C3, -1,
                        mybir.AluOpType.subtract, mybir.AluOpType.mult,
                    )
                    for it in range(2):
                        a = wpool.tile([128, 512], F32, tag="nta", bufs=2,
                                       name="nta")
                        # a = (sum*r - 2) * -1 = 2 - sum*r, fused in two ops
                        nc.vector.tensor_tensor(
                            a[:], sms[:], r[:], mybir.AluOpType.mult,
                        )
                        nc.vector.tensor_scalar(
                            a[:], a[:], 2.0, -1.0,
                            mybir.AluOpType.subtract, mybir.AluOpType.mult,
                        )
                        rn = wpool.tile([128, 512], F32, tag="rcpn", bufs=2,
                                        name="rcpn2")
                        nc.vector.tensor_tensor(
                            rn[:], r[:], a[:], mybir.AluOpType.mult,
                        )
                        r = rn
                    yt = wpool.tile([128, 512], BF16, tag="yt", bufs=4,
                                    name="yt")
                    nc.vector.tensor_tensor(
                        yt[:], yts[:], r[:], mybir.AluOpType.mult,
                    )
                    nc.gpsimd.dma_start(
                        out=bin_hh[h][slc // 2][:, (slc % 2) * 512:
                                                (slc % 2) * 512 + 512],
                        in_=yt[:],
                    )
                    if slc % 2 == 1:
                        v = slc // 2
                        nc.gpsimd.collective_compute(
                            "AllGather",
                            mybir.AluOpType.bypass,
                            replica_groups=GROUPS,
                            ins=[bin_hh[h][v][:].opt()],
                            outs=[bout_hh[h][v][:].opt()],
                        )
                drain(len(filler))

            def flat_qk(h):
                ops = []
                for mms, tq, tk in qk_closures(h):
                    ops.extend(mms)
                    ops.append(tq)
                    ops.append(tk)
                return ops

            for h in range(3):
                attention(h, flat_qk(h + 1))
            bigA_cm.__exit__(None, None, None)   # frees xta/wva SBUF for ytk

            # ---------------- phase 3: output projection --------------------
            # global head g = 4*r + h lives at bout_hh[h][v] rows r*128:+128.
            # ytk loads ride the idle sync ring; heads 0-2 preload during
            # attention(3) so phase-3 matmuls start the moment it ends.
            with (
                tc.tile_pool(name="yts", bufs=1) as ytsp,
                tc.tile_pool(name="ost", bufs=2) as ostp,
            ):
                ytk = {}

                def load_ytk(h_list):
                    for v in range(2):
                        for h in h_list:
                            for r in range(4):
                                g = 4 * r + h
                                t = ytsp.tile([128, 1024], BF16,
                                              tag=f"yt{g}_{v}",
                                              name=f"yt{g}_{v}")
                                nc.sync.dma_start(
                                    out=t[:],
                                    in_=bout_hh[h][v][r * 128:(r + 1) * 128, :],
                                )
                                ytk[(g, v)] = t

                load_ytk([0, 1, 2])

                # v0-half partial out-projection (heads 0-2) doubles as PE
                # filler during attention(3); partials park in SBUF so the
                # two psD banks recycle. m-pairs interleave to alternate
                # banks (drain overlaps fill).
                ppart = {}

                def mk_po12(v, jmp, part):
                    def go():
                        if part == 0:
                            ppart[("ps", v, jmp)] = (
                                psD.tile([128, 512], F32, tag="pq", name="poA"),
                                psD.tile([128, 512], F32, tag="pq", name="poB"),
                            )
                        pa, pb_ = ppart[("ps", v, jmp)]
                        for h in ((0, 1) if part == 0 else (2,)):
                            for r in range(4):
                                g = 4 * r + h
                                for idx, po in ((0, pa), (1, pb_)):
                                    m = v * 8 + jmp * 2 + idx
                                    nc.tensor.matmul(
                                        po[:],
                                        ytk[(g, v)][:, (m % 8) * 128:
                                                    (m % 8) * 128 + 128],
                                        woa[:, g * 512:(g + 1) * 512],
                                        start=(h == 0 and r == 0),
                                        stop=(h == 2 and r == 3),
                                    )
                        if part == 1:
                            pa, pb_ = ppart.pop(("ps", v, jmp))
                            for idx, po in ((0, pa), (1, pb_)):
                                m = v * 8 + jmp * 2 + idx
                                t = ostp.tile([128, 512], BF16, tag="ppart",
                                              bufs=8, name=f"pp{m}")
                                nc.vector.tensor_copy(t[:], po[:])
                                ppart[m] = t
                    return go

                fill3 = [mk_po12(0, jmp, part) for jmp in range(4)
                         for part in range(2)]
                attention(3, fill3)
                load_ytk([3])

                def po_finish(v, jmp):
                    """h3's contribution on top of the parked h0-2 partials,
                    then the output write for the m-pair."""
                    po = psA.tile([128, 1024], F32, tag="psc", name="po")
                    for r in range(4):
                        g = 4 * r + 3
                        for idx in range(2):
                            m = v * 8 + jmp * 2 + idx
                            nc.tensor.matmul(
                                po[:, idx * 512:idx * 512 + 512],
                                ytk[(g, v)][:, (m % 8) * 128:
                                            (m % 8) * 128 + 128],
                                woa[:, g * 512:(g + 1) * 512],
                                start=(r == 0), stop=(r == 3),
                            )
                    ot = ostp.tile([128, 1024], F32, tag="ot", name="ot")
                    for idx in range(2):
                        m = v * 8 + jmp * 2 + idx
                        nc.vector.tensor_tensor(
                            ot[:, idx * 512:idx * 512 + 512],
                            po[:, idx * 512:idx * 512 + 512],
                            ppart.pop(m)[:], mybir.AluOpType.add,
                        )
                    m0 = v * 8 + jmp * 2
                    nc.sync.dma_start(
                        out=out[m0 * 128:(m0 + 1) * 128, :],
                        in_=ot[:, 0:512],
                    )
                    nc.gpsimd.dma_start(
                        out=out[(m0 + 1) * 128:(m0 + 2) * 128, :],
                        in_=ot[:, 512:1024],
                    )

                for jmp in range(4):
                    po_finish(0, jmp)
                # v1 partials (h0-2) cover the last gather's flight time,
                # then its finishers drain.
                for jmp in range(4):
                    mk_po12(1, jmp, 0)()
                    mk_po12(1, jmp, 1)()
                for jmp in range(4):
                    po_finish(1, jmp)

    nc.finalize()
    return nc


def _host_consts():
    theta = 1.0 / (BASE ** (np.arange(0, HD, 2, dtype=np.float64)[: HD // 2] / HD))
    idx = np.arange(S, dtype=np.float64)[:, None] * theta[None, :]  # [S, 64]
    cos = np.cos(idx).astype(np.float32)
    sin = np.sin(idx).astype(np.float32)
    cosE = np.repeat(cos.T, 2, axis=0)          # [128, S]
    sinE = np.repeat(sin.T, 2, axis=0)
    sinE[0::2, :] *= -1.0                        # even rows: -sin
    P = np.zeros((128, 128), np.float32)
    P[np.arange(128), np.arange(128) ^ 1] = 1.0
    btri = np.where(
        np.arange(128)[:, None] > np.arange(128)[None, :], NEG, 0.0
    ).astype(np.float32)
    ident = np.eye(128, dtype=np.float32)
    ones2 = np.ones((128, 128), np.float32)
    return {
        "cosE": cosE.astype(NPBF16),
        "sinE": sinE.astype(NPBF16),
        "pswap": P.astype(NPBF16),
        "btri": btri.astype(NPBF16),
        "ident": ident.astype(NPBF16),
        "ones2": ones2.astype(NPBF16),
    }


def kernel(x, mask, wq, wk, wv, wo):
    global LAST_EXEC_NS, LAST_TRACE
    x = np.asarray(x, dtype=np.float32)
    wq = np.asarray(wq, dtype=np.float32)
    wk = np.asarray(wk, dtype=np.float32)
    wv = np.asarray(wv, dtype=np.float32)
    wo = np.asarray(wo, dtype=np.float32)

    consts = _host_consts()
    in_maps = []
    for core in range(NCORES):
        b, j = core // 4, core % 4
        sl = slice(j * DPC, (j + 1) * DPC)
        xT = np.ascontiguousarray(x[b].T)                       # [D, S]
        # xP[p, n*8192 + k*512 + c] = xT[k*128+p, n*512+c]
        xP = xT.reshape(KC, 128, NS, 512).transpose(1, 2, 0, 3).reshape(
            128, NS * KC * 512)
        wqT = wq[sl, :].T                                        # [D, DPC]
        wkT = wk[sl, :].T
        wvT = wv[sl, :].T
        woT = wo[sl, :].T
        # w{q,k}P[p, h*2048 + k*128 + c] = wT[k*128+p, h*128+c]
        wqP = wqT.reshape(KC, 128, HPC, 128).transpose(1, 2, 0, 3).reshape(
            128, HPC * KC * 128)
        wkP = wkT.reshape(KC, 128, HPC, 128).transpose(1, 2, 0, 3).reshape(
            128, HPC * KC * 128)
        # wvP[p, k*512 + c] = wvT[k*128+p, c]
        wvP = wvT.reshape(KC, 128, 512).transpose(1, 0, 2).reshape(
            128, KC * 512)
        # woP[p, g*512 + c] = woT[g*128+p, c]   (g = global head 0..15)
        woP = woT.reshape(H, 128, 512).transpose(1, 0, 2).reshape(
            128, H * 512)
        m = {
            "xP": np.ascontiguousarray(xP).astype(NPBF16),
            "wqP": np.ascontiguousarray(wqP).astype(NPBF16),
            "wkP": np.ascontiguousarray(wkP).astype(NPBF16),
            "wvP": np.ascontiguousarray(wvP).astype(NPBF16),
            "woP": np.ascontiguousarray(woP).astype(NPBF16),
        }
        m.update(consts)
        in_maps.append(m)

    if "nc" not in _CACHE:
        _CACHE["nc"] = _build()
    nc = _CACHE["nc"]

    trace = os.environ.get("KERNEL_TRACE", "0") == "1"
    if trace:
        trace = _install_ntff_hook()
    res = run_bass_kernel_spmd(
        nc, in_maps, core_ids=list(range(NCORES)), trace=trace,
    )
    LAST_EXEC_NS = getattr(res, "exec_time_ns", None)
    LAST_TRACE = getattr(res, "instructions_and_trace", None)

    out = np.empty((B, S, D), np.float32)
    for core in range(NCORES):
        b, j = core // 4, core % 4
        out[b, :, j * DPC:(j + 1) * DPC] = np.asarray(
            res.results[core]["out"], dtype=np.float32
        )
    return out
